# revision 1
# baseline (speedup 1.0000x reference)
"""GQA + sliding-window attention Trainium2 kernel, v2 (bf16 datapath).

Problem: B=2, S=2048, EMB=2048, 16 Q heads / 4 KV heads, head=128,
causal sliding window of 1024 (inclusive), RoPE, output projection.

Sharding: 8 cores = 2 batches x 4 KV-head groups (4 Q heads per group).

v2 changes vs baseline:
- bf16 inputs/intermediates (psum stays f32): halves DMA bytes + DVE time
- head-dim permutation so rotate-half becomes an intra-quadrant
  partition swap -> DVE stream_shuffle instead of 2 SBUF-SBUF DMAs
- one batched x DMA per seq chunk (was 16)
- exp over [128,512] psum pairs (2 k-tiles per ACT op)
- denominator matmuls col-packed 4 heads -> one PSUM tile (tile_position)
- Wo loaded up front; output projection interleaved per chunk
- output in bf16, one DMA per 128-row block
- optional For_i(iters) wrapper for dispatch-free HW timing
"""

import math

import numpy as np

S = 2048
EMB = 2048
HD = 128
QH = 4  # q heads per core (group)
NKV = 4  # kv heads total (= groups)
WINDOW = 1024
ROPE_THETA = 10000.0
SCALE = 1.0 / math.sqrt(HD)
# fp8 weight pre-scale (host multiplies Wq/Wk/Wv and their biases by WSCALE
# to lift values out of the e4m3 subnormal range; q/k/v come out scaled by
# WSCALE -- cancelled via the exp scale (q*k: WSCALE^2) and Wo/WSCALE (v))
WSCALE = 32.0
SCALE_EXP = SCALE / (WSCALE * WSCALE)

_NC_CACHE = {}


def _build_nc(loop_iters=1):
    import concourse.mybir as mybir
    import concourse.tile as tile
    from concourse import bacc
    from concourse.masks import make_identity
    from contextlib import ExitStack

    f32 = mybir.dt.float32
    bf16 = mybir.dt.bfloat16
    f8 = mybir.dt.float8e4
    AF = mybir.ActivationFunctionType
    DR = mybir.MatmulPerfMode.DoubleRow

    nc = bacc.Bacc("TRN2", target_bir_lowering=False, debug=False)

    xT = nc.dram_tensor("xT", [EMB, S], f8, kind="ExternalInput")
    xTb = nc.dram_tensor("xTb", [EMB, S], bf16, kind="ExternalInput")
    wqT = nc.dram_tensor("wqT", [EMB, QH * HD], f8, kind="ExternalInput")
    wkT = nc.dram_tensor("wkT", [EMB, HD], f8, kind="ExternalInput")
    wvT = nc.dram_tensor("wvT", [EMB, HD], bf16, kind="ExternalInput")
    woT = nc.dram_tensor("woT", [QH * HD, EMB], bf16, kind="ExternalInput")
    bq_d = nc.dram_tensor("bq", [HD, QH], f32, kind="ExternalInput")
    bk_d = nc.dram_tensor("bk", [HD, 1], f32, kind="ExternalInput")
    bv_d = nc.dram_tensor("bv", [HD, 1], f32, kind="ExternalInput")
    cos_d = nc.dram_tensor("cosT", [HD, S], bf16, kind="ExternalInput")
    sin_d = nc.dram_tensor("sinT", [HD, S], bf16, kind="ExternalInput")
    m0_d = nc.dram_tensor("mask0", [128, 128], bf16, kind="ExternalInput")
    m8_d = nc.dram_tensor("mask8", [128, 128], bf16, kind="ExternalInput")
    out_d = nc.dram_tensor("out", [S, EMB], bf16, kind="ExternalOutput")

    NE = EMB // 128  # contraction chunks
    QC = 256  # q chunk width
    NC_CHUNK = S // QC

    # rotate-half partner lives 16 partitions away within each 32-quadrant
    SHUF_MASK = [(i + 16) % 32 for i in range(32)]

    def body(tc, ctx_outer):
        with tc.tile_pool(name="const", bufs=1) as constp:
            ones_sb = constp.tile([128, 1], bf16)
            nc.vector.memset(ones_sb, 1.0)
            identf = constp.tile([128, 128], f32)
            make_identity(nc, identf)
            ident = constp.tile([128, 128], bf16)
            nc.vector.tensor_copy(ident, identf)
            m0 = constp.tile([128, 128], bf16)
            nc.sync.dma_start(m0, m0_d[:, :])
            m8 = constp.tile([128, 128], bf16)
            nc.sync.dma_start(m8, m8_d[:, :])
            bq_sb = constp.tile([HD, QH], f32)
            nc.sync.dma_start(bq_sb, bq_d[:, :])
            bk_sb = constp.tile([HD, 1], f32)
            nc.sync.dma_start(bk_sb, bk_d[:, :])
            bv_sb = constp.tile([HD, 1], f32)
            nc.sync.dma_start(bv_sb, bv_d[:, :])

            with tc.tile_pool(name="persist", bufs=1) as pers:
                q_sb = pers.tile([128, QH * S], bf16)
                k_sb = pers.tile([128, S], bf16)
                v_sb = pers.tile([128, S], bf16)

                from concourse.dve_ops import (
                    RECIP_APPROX_FAST_CONSTS,
                    RECIPROCAL_APPROX_FAST,
                )

                mmp = ctx_outer.enter_context(
                    tc.tile_pool(name="mmpsum", bufs=2, space="PSUM")
                )
                vtp = ctx_outer.enter_context(
                    tc.tile_pool(name="vtpsum", bufs=1, space="PSUM")
                )
                sp = ctx_outer.enter_context(
                    tc.tile_pool(name="scpsum", bufs=2, space="PSUM")
                )
                avp = ctx_outer.enter_context(
                    tc.tile_pool(name="avpsum", bufs=2, space="PSUM")
                )
                dp = ctx_outer.enter_context(
                    tc.tile_pool(name="dnpsum", bufs=1, space="PSUM")
                )
                with (
                    tc.tile_pool(name="phaw", bufs=1) as wp,
                    tc.tile_pool(name="xin", bufs=2) as xp,
                    tc.tile_pool(name="ptmp", bufs=3) as tpool,
                    tc.tile_pool(name="expp", bufs=8) as ep,
                    tc.tile_pool(name="nrm", bufs=2) as nr,
                    tc.tile_pool(name="outs", bufs=2) as outp,
                ):
                    # prologue DMA order matters: the SP queue is serial, so
                    # load what unblocks compute first (wk, x0, cos/sin, wv)
                    # and stream wq/wo behind it
                    wk_sb = wp.tile([128, NE * HD], f8)
                    nc.sync.dma_start(
                        wk_sb.rearrange("p (a m) -> p a m", a=NE),
                        wkT.rearrange("(a p) m -> a p m", p=128).transpose([1, 0, 2]),
                    )

                    XC = QC
                    xt_tiles = {}

                    def xt_load(c):
                        sl = slice(c * XC, (c + 1) * XC)
                        xt = xp.tile([128, NE * XC], f8, tag="xt")
                        nc.sync.dma_start(
                            xt.rearrange("p (a n) -> p a n", a=NE),
                            xT[:, sl]
                            .rearrange("(a p) n -> a p n", p=128)
                            .transpose([1, 0, 2]),
                        )
                        xtb = xp.tile([128, NE * XC], bf16, tag="xtb")
                        nc.gpsimd.dma_start(
                            xtb.rearrange("p (a n) -> p a n", a=NE),
                            xTb[:, sl]
                            .rearrange("(a p) n -> a p n", p=128)
                            .transpose([1, 0, 2]),
                        )
                        xt_tiles[c] = (xt, xtb)

                    xt_load(0)
                    cos_sb = wp.tile([HD, S], bf16)
                    nc.sync.dma_start(cos_sb, cos_d[:, :])
                    sin_sb = wp.tile([HD, S], bf16)
                    nc.sync.dma_start(sin_sb, sin_d[:, :])
                    wv_sb = wp.tile([128, NE * HD], bf16)
                    nc.sync.dma_start(
                        wv_sb.rearrange("p (a m) -> p a m", a=NE),
                        wvT.rearrange("(a p) m -> a p m", p=128).transpose([1, 0, 2]),
                    )
                    wq_sb = wp.tile([128, NE * QH * HD], f8)
                    nc.sync.dma_start(
                        wq_sb.rearrange("p (a m) -> p a m", a=NE),
                        wqT.rearrange("(a p) m -> a p m", p=128).transpose([1, 0, 2]),
                    )
                    wo_sb = wp.tile([128, QH * EMB], bf16)
                    nc.sync.dma_start(
                        wo_sb.rearrange("p (a m) -> p a m", a=QH),
                        woT.rearrange("(a p) m -> a p m", p=128).transpose([1, 0, 2]),
                    )

                    def proj(xt, w_sb, wstride, col0, bias_ap, tag):
                        # fp8 DoubleRow: contract two 128-row chunks per
                        # matmul (lhsT/rhs supply [128, 2, *] APs)
                        ps = mmp.tile([128, 512], f32, tag="mm")
                        pss = ps[:, 0:XC]
                        w_v = w_sb.rearrange("p (a m) -> p a m", a=NE)
                        xt_v = xt.rearrange("p (a n) -> p a n", a=NE)
                        for e in range(NE // 2):
                            nc.tensor.matmul(
                                pss,
                                w_v[:, 2 * e : 2 * e + 2, col0 : col0 + HD],
                                xt_v[:, 2 * e : 2 * e + 2, :],
                                start=(e == 0),
                                stop=(e == NE // 2 - 1),
                                perf_mode=DR,
                            )
                        raw = tpool.tile([128, XC], bf16, tag=tag)
                        nc.scalar.activation(raw, pss, AF.Identity, bias=bias_ap)
                        return raw

                    def projb(xtb, w_sb, bias_ap, tag):
                        # bf16 16-chunk chain (V path: fp8 too lossy)
                        ps = mmp.tile([128, 512], f32, tag="mm")
                        pss = ps[:, 0:XC]
                        for e in range(NE):
                            nc.tensor.matmul(
                                pss,
                                w_sb[:, e * HD : (e + 1) * HD],
                                xtb[:, e * XC : (e + 1) * XC],
                                start=(e == 0),
                                stop=(e == NE - 1),
                            )
                        raw = tpool.tile([128, XC], bf16, tag=tag)
                        nc.scalar.activation(raw, pss, AF.Identity, bias=bias_ap)
                        return raw

                    def rope(raw, sl, dst):
                        t1 = tpool.tile([128, XC], bf16, tag="t1")
                        t2 = tpool.tile([128, XC], bf16, tag="t2")
                        nc.vector.stream_shuffle(t2, raw, SHUF_MASK)
                        nc.vector.tensor_mul(t1, raw, cos_sb[:, sl])
                        nc.vector.tensor_mul(t2, t2, sin_sb[:, sl])
                        nc.vector.tensor_add(dst, t1, t2)

                    def out_proj(qt):
                        ot = outp.tile([128, EMB], bf16, tag="ot")
                        for ec in range(EMB // 512):
                            ops = mmp.tile([128, 512], f32, tag="mm")
                            for hh in range(QH):
                                nc.tensor.matmul(
                                    ops,
                                    q_sb[
                                        :, hh * S + qt * 128 : hh * S + (qt + 1) * 128
                                    ],
                                    wo_sb[
                                        :,
                                        hh * EMB + ec * 512 : hh * EMB + (ec + 1) * 512,
                                    ],
                                    start=(hh == 0),
                                    stop=(hh == QH - 1),
                                )
                            # split PSUM evacuation across ACT and DVE
                            if ec % 2 == 0:
                                nc.vector.tensor_copy(
                                    ot[:, ec * 512 : (ec + 1) * 512], ops
                                )
                            else:
                                nc.scalar.activation(
                                    ot[:, ec * 512 : (ec + 1) * 512], ops, AF.Copy
                                )
                        nc.gpsimd.dma_start(
                            out_d[qt * 128 : (qt + 1) * 128, :], ot
                        )

                    for c in range(NC_CHUNK):
                        sl = slice(c * XC, (c + 1) * XC)
                        xt, xtb = xt_tiles.pop(c)
                        kraw = proj(xt, wk_sb, HD, 0, bk_sb[:, 0:1], "kraw")
                        rope(kraw, sl, k_sb[:, sl])
                        vraw = projb(xtb, wv_sb, bv_sb[:, 0:1], "vraw")
                        for h in range(QH):
                            qraw = proj(
                                xt, wq_sb, QH * HD, h * HD, bq_sb[:, h : h + 1], "qraw"
                            )
                            rope(
                                qraw,
                                sl,
                                q_sb[:, h * S + c * XC : h * S + (c + 1) * XC],
                            )
                        for j in range(XC // 128):
                            tps = vtp.tile([128, 128], bf16, tag="vtr")
                            nc.tensor.transpose(
                                tps, vraw[:, j * 128 : (j + 1) * 128], ident
                            )
                            t0 = (c * XC) // 128 + j
                            nc.vector.tensor_copy(
                                v_sb[:, t0 * 128 : (t0 + 1) * 128], tps
                            )
                        # prefetch next chunk's x ahead of this chunk's
                        # attention + out-proj work
                        if c + 1 < NC_CHUNK:
                            xt_load(c + 1)

                        # -------- attention for q-chunk c, all heads --------
                        kt_lo = max(0, 2 * c - 8)
                        kts = list(range(kt_lo, 2 * c + 2))
                        n = len(kts)
                        npair = n // 2
                        dn4 = dp.tile([128, QC], f32, tag="dn")
                        # zero the 4 denominator rows; the dn matmuls below
                        # use start=False so heads don't clear each other's
                        # bank state (accumulate-onto-zero either way)
                        nc.vector.memset(dn4, 0.0)
                        for h in range(QH):
                            qsl = slice(h * S + c * QC, h * S + (c + 1) * QC)
                            ets = []
                            for p in range(npair):
                                kt0 = kts[2 * p]
                                ssp = sp.tile([128, 2 * QC], f32, tag="sc")
                                for j in range(2):
                                    nc.tensor.matmul(
                                        ssp[:, j * QC : (j + 1) * QC],
                                        k_sb[
                                            :,
                                            (kt0 + j) * 128 : (kt0 + j + 1) * 128,
                                        ],
                                        q_sb[:, qsl],
                                        start=True,
                                        stop=True,
                                    )
                                et = ep.tile([128, 2 * QC], bf16, tag="et")
                                nc.scalar.activation(et, ssp, AF.Exp, scale=SCALE_EXP)
                                for j in range(2):
                                    kt = kt0 + j
                                    for jq in range(2):
                                        d = 2 * c + jq - kt
                                        esl = et[
                                            :,
                                            j * QC + jq * 128 : j * QC + (jq + 1) * 128,
                                        ]
                                        if d < 0 or d > 8:
                                            nc.vector.memset(esl, 0.0)
                                        elif d == 0:
                                            nc.vector.tensor_mul(esl, esl, m0)
                                        elif d == 8:
                                            nc.vector.tensor_mul(esl, esl, m8)
                                ets.append(et)
                            av = avp.tile([128, QC], f32, tag="av")
                            for i, et in enumerate(ets):
                                for j in range(2):
                                    nc.tensor.matmul(
                                        av,
                                        v_sb[
                                            :,
                                            kts[2 * i + j]
                                            * 128 : (kts[2 * i + j] + 1)
                                            * 128,
                                        ],
                                        et[:, j * QC : (j + 1) * QC],
                                        start=(i == 0 and j == 0),
                                        stop=(i == npair - 1 and j == 1),
                                    )
                            # denominator for head h -> partition 32h of dn4
                            for i, et in enumerate(ets):
                                for j in range(2):
                                    nc.tensor.matmul(
                                        dn4[32 * h : 32 * h + 1, :],
                                        ones_sb,
                                        et[:, j * QC : (j + 1) * QC],
                                        start=False,
                                        stop=(i == npair - 1 and j == 1),
                                        tile_position=(0, 32 * h),
                                        skip_group_check=True,
                                    )
                            den_row = nr.tile([1, QC], f32, tag="dr")
                            nc.vector.tensor_copy(
                                den_row, dn4[32 * h : 32 * h + 1, :]
                            )
                            rec_row = nr.tile([1, QC], f32, tag="rr")
                            nc.vector._custom_dve(
                                RECIPROCAL_APPROX_FAST,
                                out=rec_row,
                                in0=den_row,
                                s0=RECIP_APPROX_FAST_CONSTS["s0"],
                                s1=RECIP_APPROX_FAST_CONSTS["s1"],
                                imm2=RECIP_APPROX_FAST_CONSTS["imm2"],
                            )
                            rec_b = nr.tile([128, QC], f32, tag="rb")
                            nc.gpsimd.partition_broadcast(rec_b, rec_row[0:1, :])
                            nc.vector.tensor_mul(q_sb[:, qsl], av, rec_b)

                        # output projection for the two 128-row blocks of
                        # chunk c (q_sb now holds normalized attention)
                        out_proj(2 * c)
                        out_proj(2 * c + 1)

    with tile.TileContext(nc) as tc, ExitStack() as ctx_outer:
        if loop_iters == 1:
            body(tc, ctx_outer)
        else:
            import concourse.mybir as mybir_

            with tc.For_i(
                0,
                loop_iters,
                1,
                hint_engines=(
                    mybir_.EngineType.PE,
                    mybir_.EngineType.Activation,
                    mybir_.EngineType.DVE,
                    mybir_.EngineType.SP,
                    mybir_.EngineType.Pool,
                ),
            ):
                with ExitStack() as ctx_inner:
                    body(tc, ctx_inner)

    nc.compile()
    return nc


def _get_nc(loop_iters=1):
    key = ("nc", loop_iters)
    if key not in _NC_CACHE:
        _NC_CACHE[key] = _build_nc(loop_iters)
    return _NC_CACHE[key]


def _get_runner(loop_iters=1):
    """Build (once) a jitted 8-core shard_map runner for the bass module."""
    key = ("runner", loop_iters)
    if key in _NC_CACHE:
        return _NC_CACHE[key]

    import jax
    from jax.experimental.shard_map import shard_map
    from jax.sharding import Mesh, NamedSharding, PartitionSpec

    import concourse.mybir as mybir
    from concourse import bass2jax

    nc = _get_nc(loop_iters)
    bass2jax.install_neuronx_cc_hook()

    partition_name = (
        nc.partition_id_tensor.name if nc.partition_id_tensor else None
    )
    in_names, out_names, out_avals, zero_outs = [], [], [], []
    for alloc in nc.m.functions[0].allocations:
        if not isinstance(alloc, mybir.MemoryLocationSet):
            continue
        name = alloc.memorylocations[0].name
        if alloc.kind == "ExternalInput":
            if name != partition_name:
                in_names.append(name)
        elif alloc.kind == "ExternalOutput":
            shape = tuple(alloc.tensor_shape)
            dtype = mybir.dt.np(alloc.dtype)
            out_avals.append(jax.core.ShapedArray(shape, dtype))
            out_names.append(name)
            zero_outs.append(np.zeros(shape, dtype))
    n_params = len(in_names)
    all_names = in_names + out_names
    if partition_name is not None:
        all_names = all_names + [partition_name]

    def _body(*args):
        operands = list(args)
        if partition_name is not None:
            operands.append(bass2jax.partition_id_tensor())
        outs = bass2jax._bass_exec_p.bind(
            *operands,
            out_avals=tuple(out_avals),
            in_names=tuple(all_names),
            out_names=tuple(out_names),
            lowering_input_output_aliases=(),
            sim_require_finite=True,
            sim_require_nnan=True,
            nc=nc,
        )
        return tuple(outs)

    n_cores = 8
    devices = jax.devices()[:n_cores]
    mesh = Mesh(np.asarray(devices), ("core",))
    spec = PartitionSpec("core")
    sharded = jax.jit(
        shard_map(
            _body,
            mesh=mesh,
            in_specs=(spec,) * (n_params + len(out_names)),
            out_specs=(spec,) * len(out_names),
            check_rep=False,
        ),
        keep_unused=True,
    )
    sharding = NamedSharding(mesh, spec)
    runner = (sharded, in_names, out_names, out_avals, zero_outs, sharding)
    _NC_CACHE[key] = runner
    return runner


def _device_inputs(in_maps, loop_iters=1):
    """Concatenate per-core inputs along axis 0 and put them on device."""
    import jax

    sharded, in_names, out_names, out_avals, zero_outs, sharding = _get_runner(
        loop_iters
    )
    arrs = []
    for name in in_names:
        cat = np.concatenate([np.asarray(m[name]) for m in in_maps], axis=0)
        arrs.append(jax.device_put(cat, sharding))
    for z in zero_outs:
        cat = np.zeros((8 * z.shape[0], *z.shape[1:]), z.dtype)
        arrs.append(jax.device_put(cat, sharding))
    return arrs


def _run_on_device(dev_args, loop_iters=1):
    sharded, in_names, out_names, out_avals, zero_outs, sharding = _get_runner(
        loop_iters
    )
    out_arrs = sharded(*dev_args)
    results = []
    for c in range(8):
        results.append(
            {
                name: np.asarray(out_arrs[i]).reshape(8, *out_avals[i].shape)[c]
                for i, name in enumerate(out_names)
            }
        )
    return results


BENCH_ITERS = 513


def bench_ns(inputs, iters=BENCH_ITERS, reps=9):
    """Per-execution device time via an on-device For_i iteration loop.

    Builds two NEFFs: the plain kernel (1 iteration) and one that runs the
    identical body `iters` times in a hardware loop (single dispatch). The
    difference of wall-clock times divided by (iters-1) cancels the
    host/dispatch overhead, leaving per-iteration device execution time
    (including the loop back-edge, i.e. slightly conservative).
    """
    import time

    import jax

    in_maps = _host_prep_from_inputs(inputs)
    dev1 = _device_inputs(in_maps, 1)
    devN = _device_inputs(in_maps, iters)
    f1 = _get_runner(1)[0]
    fN = _get_runner(iters)[0]
    # warmup (compile + first exec)
    jax.block_until_ready(f1(*dev1))
    jax.block_until_ready(fN(*devN))
    t1s, tNs = [], []
    for _ in range(reps):
        t0 = time.perf_counter()
        jax.block_until_ready(f1(*dev1))
        t1s.append(time.perf_counter() - t0)
        t0 = time.perf_counter()
        jax.block_until_ready(fN(*devN))
        tNs.append(time.perf_counter() - t0)
    t1 = min(t1s)
    tN = min(tNs)
    return max(0.0, (tN - t1)) / (iters - 1) * 1e9


def _host_prep_from_inputs(inputs):
    return _host_prep(
        np.asarray(inputs["x"], np.float32),
        np.asarray(inputs["Wq"], np.float32),
        np.asarray(inputs["bq"], np.float32),
        np.asarray(inputs["Wk"], np.float32),
        np.asarray(inputs["bk"], np.float32),
        np.asarray(inputs["Wv"], np.float32),
        np.asarray(inputs["bv"], np.float32),
        np.asarray(inputs["Wo"], np.float32),
        np.asarray(inputs["bo"], np.float32),
    )


def _perm128():
    """Head-dim permutation: partition p holds original dim PERM[p] such
    that the rotate-half partner sits 16 partitions away in-quadrant."""
    perm = np.empty(128, np.int64)
    for p in range(128):
        qd, r = p // 32, p % 32
        perm[p] = 16 * qd + r if r < 16 else 64 + 16 * qd + (r - 16)
    return perm


def _host_prep(x, Wq, bq, Wk, bk, Wv, bv, Wo, bo):
    """Build the 8 per-core input maps (bf16, permuted q/k head dims)."""
    import ml_dtypes

    bf16 = ml_dtypes.bfloat16
    f8 = ml_dtypes.float8_e4m3
    perm = _perm128()

    pos = np.arange(S, dtype=np.float64)
    inv_freq = 1.0 / (ROPE_THETA ** (np.arange(0, HD, 2, dtype=np.float64) / HD))
    freqs = pos[None, :] * inv_freq[:, None]  # (64, S)
    cos64 = np.cos(freqs)
    sin64 = np.sin(freqs)
    cosT = np.empty((HD, S), np.float32)
    sinT = np.empty((HD, S), np.float32)
    for p in range(128):
        d = perm[p]
        cosT[p] = cos64[d % 64]
        sinT[p] = -sin64[d % 64] if d < 64 else sin64[d % 64]

    ii = np.arange(128)
    mask0 = (ii[:, None] <= ii[None, :]).astype(np.float32)  # k_off <= q_off
    mask8 = (ii[:, None] >= ii[None, :]).astype(np.float32)  # k_off >= q_off

    def permute_heads(W, nheads):
        # W: (nheads*HD, EMB) -> permute the HD rows within each head
        Wr = W.reshape(nheads, HD, -1)[:, perm, :]
        return Wr.reshape(nheads * HD, -1)

    def permute_bias(b, nheads):
        return b.reshape(nheads, HD)[:, perm].reshape(nheads * HD)

    in_maps = []
    for core in range(8):
        b, g = core // NKV, core % NKV
        qs = slice(g * QH * HD, (g + 1) * QH * HD)
        ks = slice(g * HD, (g + 1) * HD)
        Wq_g = permute_heads(Wq[qs], QH) * WSCALE
        bq_g = permute_bias(bq[qs], QH) * WSCALE
        Wk_g = permute_heads(Wk[ks], 1) * WSCALE
        bk_g = permute_bias(bk[ks], 1) * WSCALE
        xTc = np.ascontiguousarray(x[b].T)
        in_maps.append(
            {
                "xT": xTc.astype(f8),
                "xTb": xTc.astype(bf16),
                "wqT": np.ascontiguousarray(Wq_g.T).astype(f8),
                "wkT": np.ascontiguousarray(Wk_g.T).astype(f8),
                "wvT": np.ascontiguousarray(Wv[ks].T).astype(bf16),
                "woT": np.ascontiguousarray(Wo[:, qs].T).astype(bf16),
                "bq": np.ascontiguousarray(bq_g.reshape(QH, HD).T),
                "bk": np.ascontiguousarray(bk_g.reshape(1, HD).T),
                "bv": np.ascontiguousarray(bv[ks].reshape(1, HD).T),
                "cosT": cosT.astype(bf16),
                "sinT": sinT.astype(bf16),
                "mask0": mask0.astype(bf16),
                "mask8": mask8.astype(bf16),
            }
        )
    return in_maps


def kernel(**inputs):
    x = np.asarray(inputs["x"], np.float32)
    bo = np.asarray(inputs["bo"], np.float32)
    in_maps = _host_prep_from_inputs(inputs)
    results = _run_on_device(_device_inputs(in_maps, 1), 1)

    out = np.empty((2, S, EMB), np.float32)
    for b in range(2):
        acc = results[b * NKV]["out"].astype(np.float32)
        for g in range(1, NKV):
            acc += results[b * NKV + g]["out"].astype(np.float32)
        out[b] = acc + bo[None, :]
    return out

